# revision 104
# baseline (speedup 1.0000x reference)
"""Trainium2 Bass kernel for nn_BidirectionalLayerFeatCosine (retrieval_knn).

Strategy: shard the 4096 query points across 8 NeuronCores (512 each); keys
are replicated.  Host rolls the key axis per core so each core's query block
is always columns 0:512 (SPMD-clean static slices).

Per core, per batch, per side: ONE wide DMA loads [feat; _; pc; pc-dup]
(fkv tile, rows 0-63 / 64-66(pc^2 target) / 67-69 / 96-98); knn is loaded in
chunks and normalized exactly in fp32 via gpsimd partition_all_reduce + ACT
sqrt + DVE reciprocal + gpsimd multiply -> khat (query side is a slice).
akv = W22@feat + Wpos@pc + b22 via one fp16 70-row matmul per chunk,
replicated to 128 partitions for the stacked gather.  Scores (cos + euclid)
stay exact fp32 on the PE; top-8 via DVE max8/find_index8; ap_gather pulls
neighbors (and a second static-index gather expands cq); the fp16 MLP uses
block-diagonal stacked weights; maxpool pair-tree; per-combo output
accumulation with one contiguous DMA emitted a combo late.
"""
import sys

for _p in ('/opt/trn_rl_repo',):
    if _p not in sys.path:
        sys.path.insert(0, _p)

import numpy as np
import concourse.bass as bass
import concourse.tile as tile
from concourse import bacc, mybir, bass_isa

F32 = mybir.dt.float32
F16 = mybir.dt.float16
I16 = mybir.dt.int16
U16 = mybir.dt.uint16
AF = mybir.ActivationFunctionType
ALU = mybir.AluOpType
ROP = bass_isa.ReduceOp

B, N, C, NS = 2, 4096, 64, 16
NCORES = 8
Q = N // NCORES           # queries per core per combo (512)
NT = Q // 128             # query tiles per combo (4)
LEAKY = 0.1
EPS = 1e-8
FKR = 99                  # fkv tile rows


def build_nc(debug_taps=False):
    nc = bacc.Bacc("TRN2", num_devices=NCORES, debug=False)

    def din(name, shape, dt=F32):
        return nc.dram_tensor(name, list(shape), dt, kind="ExternalInput").ap()

    ins = {
        'knn12': din('knn12', (B, C, 2 * N)),
        'fkv1': din('fkv1', (B, FKR, N)),
        'fkv2': din('fkv2', (B, FKR, N)),
        'w22pT': din('w22pT', (70, C), F16),
        'w11pnT': din('w11pnT', (70, C)),
        'wm1stk': din('wm1stk', (128, 128), F16),
        'wm2stk': din('wm2stk', (128, 128), F16),
        'b22': din('b22', (C, 1)),
        'bqc': din('bqc', (C, 1)),
        'bm1s': din('bm1s', (128, 1)),
        'bm2s': din('bm2s', (128, 1)),
        'id128': din('id128', (128, 128)),
        'cqidx': din('cqidx', (128, 64 * NT), I16),
    }
    # out[b, h, c, t*64+q] = feat_new[b, c, 512*core + t*128 + 64*h + q]
    out1 = nc.dram_tensor('out1', [B, 2, C, 64 * NT], F32,
                          kind="ExternalOutput").ap()
    out2 = nc.dram_tensor('out2', [B, 2, C, 64 * NT], F32,
                          kind="ExternalOutput").ap()
    taps = None
    if debug_taps:
        taps = {nm: nc.dram_tensor(nm, list(sh), dt, kind="ExternalOutput").ap()
                for nm, sh, dt in [
                    ('dbg_khat', (C, N), F32),
                    ('dbg_fkv', (FKR, N), F32),
                    ('dbg_akv2', (128, N), F32),
                    ('dbg_cq2', (128, Q), F32),
                    ('dbg_augq', (70, Q), F32),
                    ('dbg_sccos', (128, N), F32),
                    ('dbg_sceuc', (128, N), F32),
                    ('dbg_idxf', (128, 16), F32),
                    ('dbg_ag', (128, 1024), F32),
                    ('dbg_cqs', (128, 1024), F32),
                    ('dbg_h2', (128, 1024), F16),
                ]}

    with tile.TileContext(nc) as tc:
        _body(tc, ins, out1, out2, taps)
    nc.compile()
    return nc


def _body(tc, ins, out1, out2, taps=None):
    nc = tc.nc
    from contextlib import ExitStack
    ctx = ExitStack()

    pool = lambda name, bufs, space='SBUF': ctx.enter_context(
        tc.tile_pool(name=name, bufs=bufs, space=space))

    consts = pool('consts', 1)
    inp = pool('inputs', 2)        # fkv tiles (rotate across sides/batches)
    prep = pool('prep', 2)         # chunked scratch for normalization
    keyp = pool('keyprep', 2)      # khat / akv2 (both sides live)
    f16p = pool('f16', 1)
    qp = pool('qside', 2)          # cq2 / augq per combo
    scp = pool('scores', 2)        # [128, 4096] score rows
    idxp = pool('idx', 2)          # vals/idx tiles per tile
    mlpp = pool('mlp', 1)
    outp = pool('out', 2)

    sc_ps = ctx.enter_context(tc.tile_pool(name='sc_ps', bufs=2, space='PSUM'))
    mlp_ps = ctx.enter_context(tc.tile_pool(name='mlp_ps', bufs=1, space='PSUM'))
    p64_ps = ctx.enter_context(tc.tile_pool(name='p64_ps', bufs=1, space='PSUM'))
    tp_ps = ctx.enter_context(tc.tile_pool(name='tp_ps', bufs=1, space='PSUM'))

    # ---- constants ----
    def cload(name, shape, dt=F32):
        t = consts.tile(list(shape), dt, tag=name, name=name)
        nc.sync.dma_start(t[:], ins[name])
        return t

    # ---------- per (batch, side) key prep ----------
    CH = 1024                      # normalization chunk width

    def khat_start(bi):
        # Both sides column-stacked [64, 2N]; side 2 (the first combo's key
        # side) loads first so its chain can run first.
        knn = prep.tile([C, 2 * N], F32, tag='knn', name='knn', bufs=1)
        nc.sync.dma_start(knn[:, N:2 * N], ins['knn12'][bi][:, N:2 * N])
        nc.sync.dma_start(knn[:, 0:N], ins['knn12'][bi][:, 0:N])
        return knn

    def khat_chain(bi, knn):
        # khat = knn / sqrt(colsum(knn^2) + eps), exact fp32 matching the
        # reference's rounding (sqrt of biased sum, then reciprocal —
        # reordering these flips near-tie neighbor selections).
        # Emits side 2 + the first side-1 chunk now (all the first combo
        # needs); the remaining side-1 chunks are returned as deferred
        # steps, popped inside the first combo's tile loop.
        khatC = keyp.tile([C, 2 * N], F32, tag='khatC', name='khatC',
                          bufs=1)
        half = N // CH

        def chunk(j):
            sl = slice(j * CH, (j + 1) * CH)
            ksq = prep.tile([C, CH], F32, tag='scrA', name='ksq')
            if bi == 0:
                nc.scalar.activation(ksq[:], knn[:, sl], AF.Square)
            else:
                nc.gpsimd.tensor_tensor(ksq[:], knn[:, sl], knn[:, sl],
                                        op=ALU.mult)
            ssb = prep.tile([C, CH], F32, tag='scrB', name='ssb')
            nc.gpsimd.partition_all_reduce(ssb[:], ksq[:], channels=C,
                                           reduce_op=ROP.add)
            nc.scalar.activation(ssb[:], ssb[:], AF.Sqrt, bias=eps128[0:C])
            rinv = prep.tile([C, CH], F32, tag='scrA', name='rinv')
            nc.vector.reciprocal(rinv[:], ssb[:])
            nc.gpsimd.tensor_tensor(khatC[:, sl], knn[:, sl], rinv[:],
                                    op=ALU.mult)

        for j in list(range(half, 2 * half)) + list(range(half)):
            chunk(j)
        return khatC, []

    def fkv_load(bi, side, defer_sq=False):
        fkv_d = ins['fkv1'] if side == 1 else ins['fkv2']
        # fkv rows: 0-63 feat, 64-66 pc^2 (computed), 67-69 pc, 96-98 pc.
        fkv = inp.tile([FKR, N], F32, tag='fkv', name='fkv')
        nc.sync.dma_start(fkv[:], fkv_d[bi])
        if not defer_sq:
            pc_sq(fkv)
        return fkv

    def pc_sq(fkv):
        for j in range(4):
            sl = slice(j * 1024, (j + 1) * 1024)
            nc.scalar.activation(fkv[C:C + 3, sl], fkv[96:99, sl],
                                 AF.Square)

    def akv_build(sd_side):
        # akv2 = [W22; 0; Wpos] @ fkv[0:70] + b22, replicated to 128 parts
        fkv = sd_side['fkv']
        akv2 = keyp.tile([128, N], F32, tag='akv2', name='akv2')
        for kb in range(N // 512):
            sl = slice(kb * 512, (kb + 1) * 512)
            fkv16 = f16p.tile([70, 512], F16, tag='fkv16', name='fkv16',
                              bufs=2)
            nc.gpsimd.tensor_copy(fkv16[:], fkv[0:70, sl])
            ps = p64_ps.tile([C, 512], F32, tag='p64', name='akv_ps')
            nc.tensor.matmul(ps[:], lhsT=w22pT[:], rhs=fkv16[:],
                             start=True, stop=True)
            nc.scalar.activation(akv2[0:C, sl], ps[:], AF.Identity,
                                 bias=b22[:])
            nc.sync.dma_start(akv2[C:128, sl], akv2[0:C, sl])
        sd_side['akv2'] = akv2

    def key_prep_batch(bi, knn):
        khatC, ksteps = khat_chain(bi, knn)
        fkv1 = fkv_load(bi, 1, defer_sq=True)
        fkv2 = fkv_load(bi, 2)
        sd = {1: dict(fkv=fkv1, khat=khatC, koff=0),
              2: dict(fkv=fkv2, khat=khatC, koff=N)}
        pc_sq(fkv1)
        # akv_build for both sides is deferred into the first combo's tile
        # loop — akv2 is first needed by the gathers, well after scores.
        sd['ksteps'] = ksteps
        return sd

    # ---------- per-combo query prep ----------
    def query_prep(sd_q):
        fkv = sd_q['fkv']
        # cq = W11@feat_q - Wpos@pc_q + (b11 + bpos), stacked to 128 rows
        cq2 = qp.tile([128, Q], F32, tag='cq2', name='cq2')
        ps = p64_ps.tile([C, 512], F32, tag='p64', name='cq_ps')
        nc.tensor.matmul(ps[:, :Q], lhsT=w11pnT[:], rhs=fkv[0:70, 0:Q],
                         start=True, stop=True)
        nc.scalar.activation(cq2[0:C, :], ps[:, :Q], AF.Identity,
                             bias=bqc[:])
        nc.scalar.activation(cq2[C:128, :], ps[:, :Q], AF.Identity,
                             bias=bqc[:])
        # augq rows 64-69: [-0.5 x3, qx, qy, qz] — contracts with fkv rows
        # 64-69 = [x^2, y^2, z^2, x, y, z]: score = q.k - 0.5|k|^2.
        augq = qp.tile([70, Q], F32, tag='augq', name='augq')
        nc.scalar.activation(augq[C:C + 3, :], fkv[C:C + 3, 0:Q],
                             AF.Copy, scale=0.0, bias=-0.5)
        nc.sync.dma_start(augq[C + 3:C + 6, :], fkv[C + 3:C + 6, 0:Q])
        return dict(cq2=cq2, augq=augq)

    # ---------- tile stages ----------
    def tile_scores(cb):
        sd_q, sd_k, t = cb['q'], cb['k'], cb['t']
        tsl = slice(t * 128, (t + 1) * 128)
        qtsl = slice(sd_q['koff'] + t * 128, sd_q['koff'] + (t + 1) * 128)
        khat = sd_q['khat']
        ko = sd_k['koff']
        fkv_k = sd_k['fkv']
        augq = cb['qd']['augq']

        sc_cos = scp.tile([128, N], F32, tag='sc', name='sc_cos')
        for j in range(N // 1024):
            ps = sc_ps.tile([128, 1024], F32, tag='sc_ps', name='sc_ps')
            for h in range(2):
                sl = slice(ko + j * 1024 + h * 512,
                           ko + j * 1024 + (h + 1) * 512)
                nc.tensor.matmul(ps[:, h * 512:(h + 1) * 512],
                                 lhsT=khat[:, qtsl], rhs=khat[:, sl],
                                 start=True, stop=True)
            nc.scalar.activation(sc_cos[:, j * 1024:(j + 1) * 1024],
                                 ps[:], AF.Copy)
        sc_euc = scp.tile([128, N], F32, tag='sc', name='sc_euc')
        for j in range(N // 1024):
            ps = sc_ps.tile([128, 1024], F32, tag='sc_ps', name='sc_ps')
            for h in range(2):
                sl = slice(j * 1024 + h * 512, j * 1024 + (h + 1) * 512)
                nc.tensor.matmul(ps[:, h * 512:(h + 1) * 512],
                                 lhsT=augq[C:C + 6, tsl],
                                 rhs=fkv_k[C:C + 6, sl],
                                 start=True, stop=True)
            nc.scalar.activation(sc_euc[:, j * 1024:(j + 1) * 1024],
                                 ps[:], AF.Copy)
        cb['sc_cos'], cb['sc_euc'] = sc_cos, sc_euc
        if taps is not None and cb['ci'] == 0 and t == 0:
            nc.sync.dma_start(taps['dbg_khat'], khat[:, ko:ko + N])
            nc.sync.dma_start(taps['dbg_fkv'], fkv_k[:])
            nc.sync.dma_start(taps['dbg_akv2'], sd_k['akv2'][:])
            nc.sync.dma_start(taps['dbg_cq2'], cb['qd']['cq2'][:])
            nc.sync.dma_start(taps['dbg_augq'], augq[:])
            nc.sync.dma_start(taps['dbg_sccos'], sc_cos[:])
            nc.sync.dma_start(taps['dbg_sceuc'], sc_euc[:])

    def tile_topk(cb):
        vals = idxp.tile([128, 16], F32, tag='vals', name='vals')
        idxu = idxp.tile([128, 16], U16, tag='idxu', name='idxu')
        nc.vector.max(vals[:, 0:8], cb['sc_cos'][:])
        nc.vector.max_index(idxu[:, 0:8], vals[:, 0:8], cb['sc_cos'][:])
        nc.vector.max(vals[:, 8:16], cb['sc_euc'][:])
        nc.vector.max_index(idxu[:, 8:16], vals[:, 8:16], cb['sc_euc'][:])
        cb['idxu'] = idxu

    def tile_post(cb):
        sd_k, t = cb['k'], cb['t']
        cq2 = cb['qd']['cq2']
        akv2 = sd_k['akv2']

        # ---- index transpose to gather layout ----
        idxf = idxp.tile([128, 16], F32, tag='idxf', name='idxf')
        nc.vector.tensor_copy(idxf[:], cb['idxu'][:])
        pst = tp_ps.tile([16, 128], F32, tag='tp', name='pst')
        nc.tensor.matmul(pst[:], lhsT=idxf[:], rhs=id128[:],
                         start=True, stop=True)
        idxrow = idxp.tile([16, 128], I16, tag='idxrow', name='idxrow')
        nc.scalar.activation(idxrow[:], pst[:], AF.Copy)
        idxT = idxp.tile([128, 64], I16, tag='idxT', name='idxT')
        for h in range(2):
            src = idxrow[:, h * 64:(h + 1) * 64]
            for g in range(4):
                b = h * 64 + g * 16
                nc.sync.dma_start(idxT[b:b + 16, :], src)

        # ---- gathers: neighbors + per-query cq expansion ----
        ag = mlpp.tile([128, 1024], F32, tag='ag', name='ag')
        nc.gpsimd.ap_gather(ag[:], akv2[:], idxT[:], channels=128,
                            num_elems=N, d=1, num_idxs=1024)
        cqs = mlpp.tile([128, 1024], F32, tag='cqs', name='cqs')
        nc.gpsimd.ap_gather(cqs[:], cq2[:], cqidx[:, t * 64:(t + 1) * 64],
                            channels=128, num_elems=Q, d=1, num_idxs=1024)
        if taps is not None and cb['ci'] == 0 and t == 0:
            nc.sync.dma_start(taps['dbg_idxf'], idxf[:])
            nc.sync.dma_start(taps['dbg_ag'], ag[:])
            nc.sync.dma_start(taps['dbg_cqs'], cqs[:])

        # ---- layer 0: add cq, leaky (-> fp16) ----
        nc.gpsimd.tensor_tensor(ag[:], ag[:], cqs[:], op=ALU.add)
        n1 = mlpp.tile([128, 1024], F16, tag='n1', name='n1')
        nc.scalar.activation(n1[:], ag[:], AF.Prelu, alpha=LEAKY)

        # ---- layers 1, 2 (fp16, stacked weights) ----
        cur = n1
        for li, (w, bias) in enumerate(((wm1stk, bm1s), (wm2stk, bm2s))):
            ps = mlp_ps.tile([128, 1024], F32, tag='mlp', name='mlp_ps')
            for h in range(2):
                hs = slice(h * 512, (h + 1) * 512)
                nc.tensor.matmul(ps[:, hs], lhsT=w[:], rhs=cur[:, hs],
                                 start=True, stop=True)
            ht = mlpp.tile([128, 1024], F16, tag=f'h{li + 1}',
                           name=f'h{li + 1}')
            nc.scalar.activation(ht[:], ps[:], AF.Prelu, bias=bias[:],
                                 alpha=LEAKY)
            cur = ht
        if taps is not None and cb['ci'] == 0 and t == 0:
            nc.sync.dma_start(taps['dbg_h2'], cur[:])

        # ---- maxpool over NS neighbors (DVE half-pair tree: packed
        # contiguous slot runs keep the fp16 2x mode) ----
        width = NS
        while width > 1:
            w2 = width // 2
            if w2 == 1:
                nxt = cb['outc']
                dst = nxt[:, t * 64:(t + 1) * 64].rearrange(
                    'c (q k) -> c q k', k=1)
            else:
                nxt = mlpp.tile([128, 64 * w2], F16, tag=f'mp{w2}',
                                name=f'mp{w2}')
                dst = nxt[:].rearrange('c (q k) -> c q k', k=w2)
            v = cur[:].rearrange('c (q k) -> c q k', k=width)
            nc.vector.tensor_tensor(dst, v[:, :, 0:w2], v[:, :, w2:width],
                                    op=ALU.max)
            cur = nxt
            width = w2

    def emit_out(cb):
        # one contiguous DMA per combo: [128, 256] -> out[b, h, c, :]
        outap = cb['outap']
        base = outap.offset + cb['bi'] * 2 * C * 64 * NT
        dst = bass.AP(outap.tensor, base,
                      [[C * 64 * NT, 2], [64 * NT, C], [1, 64 * NT]])
        nc.sync.dma_start(dst, cb['outc'][:])

    # ---------- main loop (software pipelined, incl. batch-level) ----------
    pending = None
    pending_out = None
    ci = 0
    # knn loads first (critical path), then the constant loads
    knn_next = khat_start(0)
    w22pT = cload('w22pT', (70, C), F16)
    w11pnT = cload('w11pnT', (70, C))
    wm1stk = cload('wm1stk', (128, 128), F16)
    wm2stk = cload('wm2stk', (128, 128), F16)
    b22 = cload('b22', (C, 1))
    bqc = cload('bqc', (C, 1))
    bm1s = cload('bm1s', (128, 1))
    bm2s = cload('bm2s', (128, 1))
    id128 = cload('id128', (128, 128))
    cqidx = cload('cqidx', (128, 64 * NT), I16)
    eps128 = consts.tile([128, 1], F32, tag='eps128', name='eps128')
    nc.vector.memset(eps128[:], EPS)
    sd_next = key_prep_batch(0, knn_next)
    for bi in range(B):
        sd = sd_next
        for outap, qs, ks in ((out1, 1, 2), (out2, 2, 1)):
            qd = query_prep(sd[qs])
            if ci == 0:
                # warm the PE p-state so the first score matmuls run at
                # full clock
                for _ in range(14):
                    wps = sc_ps.tile([128, 1024], F32, tag='sc_ps',
                                     name='warm_ps')
                    nc.tensor.matmul(wps[:, 0:128], lhsT=id128[:],
                                     rhs=id128[:], start=True, stop=True)
            outc = outp.tile([128, 64 * NT], F32, tag='outc', name='outc')
            for t in range(NT):
                cb = dict(q=sd[qs], k=sd[ks], qd=qd, t=t, bi=bi,
                          outap=outap, outc=outc, ci=ci)
                tile_scores(cb)
                if qs == 1 and sd['ksteps']:
                    sd['ksteps'].pop(0)()
                if qs == 1 and t == 0 and 'akv2' not in sd[2]:
                    akv_build(sd[2])
                if qs == 1 and t == 1 and 'akv2' not in sd[1]:
                    akv_build(sd[1])
                if qs == 2 and t == 0 and bi + 1 < B:
                    knn_next = khat_start(bi + 1)
                if pending is not None:
                    tile_post(pending)
                if pending_out is not None and ci > pending_out[0] \
                        and t == 2:
                    emit_out(pending_out[1])
                    pending_out = None
                tile_topk(cb)
                pending = cb
            pending_out = (ci, cb)
            ci += 1
        if bi + 1 < B:
            # prefetch next batch's key prep behind this batch's tail
            sd_next = key_prep_batch(bi + 1, knn_next)
    tile_post(pending)
    emit_out(pending_out[1])
    ctx.close()


# ======================= host side =======================

_CACHED = {}


def _get_nc():
    if 'nc' not in _CACHED:
        _CACHED['nc'] = build_nc()
    return _CACHED['nc']


def make_in_maps(pc1, pc2, feat1, feat2, knn1, knn2,
                 W_t11, b_t11, W_t22, b_t22, W_pos, b_pos,
                 W_m1, b_m1, W_m2, b_m2):
    f32, f16 = np.float32, np.float16
    W_t11 = np.asarray(W_t11, f32); W_t22 = np.asarray(W_t22, f32)
    W_pos = np.asarray(W_pos, f32)
    W_m1 = np.asarray(W_m1, f32); W_m2 = np.asarray(W_m2, f32)

    z3 = np.zeros((3, C), f32)
    w22pT = np.vstack([W_t22.T, z3, W_pos.T]).astype(f16)        # [70, 64]
    w11pnT = np.vstack([W_t11.T, z3, -W_pos.T]).astype(f32)      # [70, 64]
    z = np.zeros((C, C), f32)
    wm1stk = np.block([[W_m1.T, z], [z, W_m1.T]]).astype(f16)    # [128,128]
    wm2stk = np.block([[W_m2.T, z], [z, W_m2.T]]).astype(f16)
    b22 = np.asarray(b_t22, f32).reshape(C, 1)
    bqc = (np.asarray(b_t11, f32) + np.asarray(b_pos, f32)).reshape(C, 1)
    bm1s = np.tile(np.asarray(b_m1, f32).reshape(C, 1), (2, 1))
    bm2s = np.tile(np.asarray(b_m2, f32).reshape(C, 1), (2, 1))

    # cqidx[16g+s, t*64+q] = t*128 + 64*(g>=4) + q: groups 0-3 expand the
    # first 64 queries of tile t, groups 4-7 the second 64.
    cqidx = np.zeros((128, 64 * NT), np.int16)
    for g in range(8):
        h = g // 4
        for t in range(NT):
            cqidx[16 * g:16 * (g + 1), t * 64:(t + 1) * 64] = \
                t * 128 + 64 * h + np.arange(64, dtype=np.int16)[None, :]

    def build_fkv(feat, pc):
        b, _, n = feat.shape
        fkv = np.zeros((b, FKR, n), f32)
        fkv[:, 0:C] = feat
        fkv[:, C + 3:C + 6] = pc
        fkv[:, 96:99] = pc
        return fkv

    fkv1 = build_fkv(np.asarray(feat1, f32), np.asarray(pc1, f32))
    fkv2 = build_fkv(np.asarray(feat2, f32), np.asarray(pc2, f32))
    knn1 = np.asarray(knn1, f32)
    knn2 = np.asarray(knn2, f32)

    base = {
        'w22pT': w22pT, 'w11pnT': w11pnT,
        'wm1stk': wm1stk, 'wm2stk': wm2stk,
        'b22': b22, 'bqc': bqc, 'bm1s': bm1s, 'bm2s': bm2s,
        'id128': np.eye(128, dtype=f32),
        'cqidx': cqidx,
    }
    in_maps = []
    for c in range(NCORES):
        m = dict(base)
        r = -c * Q
        m['knn12'] = np.ascontiguousarray(np.concatenate(
            [np.roll(knn1, r, axis=2), np.roll(knn2, r, axis=2)], axis=2))
        m['fkv1'] = np.ascontiguousarray(np.roll(fkv1, r, axis=2))
        m['fkv2'] = np.ascontiguousarray(np.roll(fkv2, r, axis=2))
        in_maps.append(m)
    return in_maps


def _unstack_out(res, name):
    # per-core out [B, 2, C, 256] -> [B, C, 512] block, concat on queries
    blocks = []
    for c in range(NCORES):
        o = res.results[c][name]           # [B, 2, C, 256]
        o = o.reshape(B, 2, C, NT, 64)     # [b, h, c, t, q]
        o = o.transpose(0, 2, 3, 1, 4)     # [b, c, t, h, q]
        blocks.append(o.reshape(B, C, Q))
    return np.concatenate(blocks, axis=2)


def kernel(pc1, pc2, feat1, feat2, knn1, knn2,
           W_t11, b_t11, W_t22, b_t22, W_pos, b_pos,
           W_m1, b_m1, W_m2, b_m2):
    from concourse.bass_utils import run_bass_kernel_spmd
    nc = _get_nc()
    in_maps = make_in_maps(pc1, pc2, feat1, feat2, knn1, knn2,
                           W_t11, b_t11, W_t22, b_t22, W_pos, b_pos,
                           W_m1, b_m1, W_m2, b_m2)
    res = run_bass_kernel_spmd(nc, in_maps, core_ids=list(range(NCORES)))
    return _unstack_out(res, 'out1'), _unstack_out(res, 'out2')
